# revision 8
# baseline (speedup 1.0000x reference)
"""Trainium2 Bass kernel for nn_AggregationLayer (segment_reduce).

Strategy: shard the spatial HW dimension (480*480 = 230400 pixels) across 8
NeuronCores (28800 pixels each). Each core:
  - computes partial masked sums for ALL 64 instances x (8 map channels x 8
    samples + ones) via PE matmul: maskT[hw,64].T @ maps[hw,130] accumulated
    over 225 chunks of 128 pixels (PSUM fp32). Maps are sent as bf16 hi+lo
    pairs (fp32 split) so the bf16 matmul reaches ~fp32 accuracy; masks are
    0/1 which bf16 represents exactly.
  - computes the dense xy_masked product on DVE with broadcast access
    patterns (mask bcast over channel, xy bcast over instance).
Host: pre-arranges inputs per core (pure layout transforms), then sums the 8
partial results, applies mean / L2-normalize / exp (64x9 scalars), and
reassembles xy_masked. No cross-device communication needed.

The 225 chunks are processed in asymmetric blocks (small first block so
compute starts early; small last block to shrink the serial tail), and each
block's store is split into two half-tiles so the output DMA of half 0
overlaps the product of half 1.
"""

import os
from contextlib import ExitStack

import numpy as np
import ml_dtypes

import concourse.bass as bass
import concourse.bacc as bacc
import concourse.tile as tile
import concourse.mybir as mybir
from concourse.bass_utils import run_bass_kernel_spmd

# Problem shape (hardcoded per contract)
B = 8            # samples
N = 64           # instances
NH = N // 2      # half split for stores
H = W = 480
HW = H * W       # 230400
NCORES = 8
HW_CORE = HW // NCORES          # 28800
P = 128                          # partitions
CHUNKS = HW_CORE // P            # 225
BLOCK_SIZES = [30, 55, 55, 55, 20, 10]
assert sum(BLOCK_SIZES) == CHUNKS
BLOCK_OFFS = np.concatenate([[0], np.cumsum(BLOCK_SIZES)[:-1]]).tolist()
NRED = B * 8 + 1                 # 65: (8 samples x [quat4, scales3, z1]) + ones
NCOLS = 2 * NRED                 # 130: hi + lo
NXY = B * 2                      # 16 xy rows
EPS = 1e-12

LAST_RESULT = None  # BassKernelResults of the most recent run (for test harness)

_PROGRAM_CACHE = {}


def _build_program(counts, starts):
    """Build the SPMD Bass program. counts/starts: per-sample instance group
    boundaries in the sorted instance order (baked into DVE APs)."""
    nc = bacc.Bacc(
        "TRN2",
        target_bir_lowering=False,
        debug=False,
        enable_asserts=False,
        num_devices=NCORES,
    )
    bf16 = mybir.dt.bfloat16
    f32 = mybir.dt.float32
    fp8 = mybir.dt.float8e4

    d_mask = nc.dram_tensor("maskt", [P, N * CHUNKS], bf16, kind="ExternalInput").ap()
    d_red = nc.dram_tensor("mapsred", [P, NCOLS * CHUNKS], bf16, kind="ExternalInput").ap()
    d_xy = nc.dram_tensor("mapsxy", [P, NXY * CHUNKS], f32, kind="ExternalInput").ap()
    d_part = nc.dram_tensor("partials", [N, NCOLS], f32, kind="ExternalOutput").ap()
    d_out = nc.dram_tensor("xyout", [P, N * 2 * CHUNKS], f32, kind="ExternalOutput").ap()

    # (sample, n-range) pieces per store half: list of (s, st, cnt) with the
    # sorted-instance range clipped to each half
    halves = []
    for h in range(2):
        lo, hi = h * NH, (h + 1) * NH
        pieces = []
        for s in range(B):
            st, cnt = int(starts[s]), int(counts[s])
            a, b_ = max(st, lo), min(st + cnt, hi)
            if a < b_:
                pieces.append((s, a, b_ - a))
        halves.append(pieces)

    with tile.TileContext(nc) as tc:
        with ExitStack() as ctx:
            pool = ctx.enter_context(tc.tile_pool(name="work", bufs=3))
            ppool = ctx.enter_context(tc.tile_pool(name="acc", bufs=1, space="PSUM"))

            t_psum = ppool.tile([N, NCOLS], f32)
            nblocks = len(BLOCK_SIZES)
            for b, (cpb, off) in enumerate(zip(BLOCK_SIZES, BLOCK_OFFS)):
                t_mask = pool.tile([P, N * cpb], bf16, tag="mask")
                nc.sync.dma_start(
                    t_mask[:].rearrange("p (n j) -> p n j", n=N),
                    d_mask[:, N * off:N * (off + cpb)].rearrange("p (n j) -> p n j", n=N),
                )
                t_xy = pool.tile([P, NXY * cpb], f32, tag="xy")
                nc.sync.dma_start(t_xy[:], d_xy[:, NXY * off:NXY * (off + cpb)])
                t_red = pool.tile([P, NCOLS * cpb], bf16, tag="red")
                nc.sync.dma_start(t_red[:], d_red[:, NCOLS * off:NCOLS * (off + cpb)])

                # mask is (n j) layout; lhsT per chunk j reads columns at stride cpb
                mask_mm = t_mask[:].rearrange("p (n j) -> p j n", n=N)
                for j in range(cpb):
                    nc.tensor.matmul(
                        t_psum[:],
                        mask_mm[:, j, :],
                        t_red[:, j * NCOLS:(j + 1) * NCOLS],
                        start=(b == 0 and j == 0),
                        stop=(b == nblocks - 1 and j == cpb - 1),
                    )

                mask_r = t_mask[:].rearrange("p (n j) -> p n j", n=N)
                xy_r = t_xy[:].rearrange("p (g j) -> p g j", j=cpb)
                for h in range(2):
                    t_out = pool.tile([P, NH * 2 * cpb], f32, tag=f"out{h}")
                    out_r = t_out[:].rearrange("p (n c j) -> p n c j", n=NH, c=2)
                    for s, a, cnt in halves[h]:
                        ah = a - h * NH
                        o = out_r[:, ah:ah + cnt, :, :]
                        m = mask_r[:, a:a + cnt, :].unsqueeze(2).broadcast_to([P, cnt, 2, cpb])
                        x = xy_r[:, 2 * s:2 * s + 2, :].unsqueeze(1).broadcast_to([P, cnt, 2, cpb])
                        nc.vector.tensor_mul(o, m, x)
                    base = 2 * (N * off + h * NH * cpb)
                    nc.sync.dma_start(d_out[:, base:base + NH * 2 * cpb], t_out[:])

            t_part = pool.tile([N, NCOLS], f32, tag="part")
            nc.vector.tensor_copy(t_part[:], t_psum[:])
            nc.sync.dma_start(d_part[:], t_part[:])

    nc.compile()
    return nc


def _blocked_cols(arr_pcj, width):
    """arr_pcj: [P, rows, 225] -> [P, sum(rows*cpb)] with per-block (row, j)
    column layout, blocks in BLOCK_SIZES order. width = rows."""
    outs = []
    for cpb, off in zip(BLOCK_SIZES, BLOCK_OFFS):
        blk = arr_pcj[:, :, off:off + cpb]              # [P, rows, cpb]
        outs.append(blk.reshape(P, width * cpb))
    return np.concatenate(outs, axis=1)


def kernel(instance_masks, sample_ids, quaternion, scales, xy, z):
    global LAST_RESULT
    instance_masks = np.asarray(instance_masks, dtype=np.float32)
    sample_ids = np.asarray(sample_ids).astype(np.int64)
    quaternion = np.asarray(quaternion, dtype=np.float32)
    scales = np.asarray(scales, dtype=np.float32)
    xy = np.asarray(xy, dtype=np.float32)
    z = np.asarray(z, dtype=np.float32)

    # --- host-side layout prep -------------------------------------------
    order = np.argsort(sample_ids, kind="stable")          # n_sorted -> n_orig
    counts = np.bincount(sample_ids, minlength=B)
    starts = np.concatenate([[0], np.cumsum(counts)[:-1]])

    masks_s = instance_masks[order].reshape(N, HW)
    # [core, p, n, 225chunks]
    mask_c = masks_s.astype(ml_dtypes.bfloat16).reshape(N, NCORES, CHUNKS, P)
    mask_c = np.ascontiguousarray(mask_c.transpose(1, 3, 0, 2))   # [core, P, N, 225]

    # reduce maps: [s, ch8, HW] with ch = quat0..3, scale0..2, z -> col s*8+ch
    red = np.concatenate(
        [quaternion.reshape(B, 4, HW), scales.reshape(B, 3, HW), z.reshape(B, 1, HW)],
        axis=1,
    ).reshape(B * 8, HW)
    red65 = np.concatenate([red, np.ones((1, HW), np.float32)], axis=0)  # [65, HW]
    hi = red65.astype(ml_dtypes.bfloat16)
    lo = (red65 - hi.astype(np.float32)).astype(ml_dtypes.bfloat16)
    red130 = np.concatenate([hi, lo], axis=0)              # [130, HW] bf16
    # per-chunk layout is (j, col130): [core, p, 225, 130] -> treat rows=(j col)
    red_c = red130.reshape(NCOLS, NCORES, CHUNKS, P)
    red_c = np.ascontiguousarray(red_c.transpose(1, 3, 2, 0))     # [core, P, 225, 130]

    xy_c = xy.reshape(NXY, NCORES, CHUNKS, P)
    xy_c = np.ascontiguousarray(xy_c.transpose(1, 3, 0, 2))       # [core, P, 16, 225]

    in_maps = []
    for k in range(NCORES):
        maskt = _blocked_cols(mask_c[k], N)
        # red blocks: layout (j, col): reshape [P, 225, 130] slice then flatten
        red_blocks = []
        for cpb, off in zip(BLOCK_SIZES, BLOCK_OFFS):
            red_blocks.append(red_c[k][:, off:off + cpb, :].reshape(P, cpb * NCOLS))
        mapsred = np.concatenate(red_blocks, axis=1)
        mapsxy = _blocked_cols(xy_c[k], NXY)
        in_maps.append({
            "maskt": np.ascontiguousarray(maskt),
            "mapsred": np.ascontiguousarray(mapsred),
            "mapsxy": np.ascontiguousarray(mapsxy),
        })

    # --- build + run ------------------------------------------------------
    key = (tuple(counts.tolist()),)
    if key not in _PROGRAM_CACHE:
        _PROGRAM_CACHE[key] = _build_program(counts, starts)
    nc = _PROGRAM_CACHE[key]

    trace = bool(int(os.environ.get("KERNEL_TRACE", "0")))
    res = run_bass_kernel_spmd(nc, in_maps, core_ids=list(range(NCORES)), trace=trace)
    LAST_RESULT = res

    # --- host-side combine ------------------------------------------------
    parts = np.stack([res.results[k]["partials"] for k in range(NCORES)])  # [8, N, 130]
    S = parts.astype(np.float64).sum(axis=0)
    S = S[:, :NRED] + S[:, NRED:]                                          # [N, 65]

    sid_sorted = sample_ids[order]
    msize = S[:, B * 8]                                                    # [N]
    cols = sid_sorted[:, None] * 8 + np.arange(8)[None, :]                 # [N, 8]
    sums = np.take_along_axis(S[:, :B * 8], cols, axis=1)                  # [N, 8]
    means = sums / msize[:, None]
    q = means[:, :4]
    qn = np.linalg.norm(q, axis=1, keepdims=True)
    q_agg_s = (q / np.maximum(qn, EPS)).astype(np.float32)
    s_agg_s = means[:, 4:7].astype(np.float32)
    z_agg_s = np.exp(means[:, 7:8]).astype(np.float32)

    inv = np.empty(N, dtype=np.int64)
    inv[order] = np.arange(N)
    q_agg = q_agg_s[inv]
    s_agg = s_agg_s[inv]
    z_agg = z_agg_s[inv]

    # xyout cols: per block, (n, c, j) with n split in halves handled by layout
    X = np.stack([res.results[k]["xyout"] for k in range(NCORES)])  # [8, P, N*2*225]
    xy_masked_s = np.empty((N, 2, NCORES, CHUNKS, P), np.float32)
    for cpb, off in zip(BLOCK_SIZES, BLOCK_OFFS):
        base = 2 * N * off
        blk = X[:, :, base:base + N * 2 * cpb].reshape(NCORES, P, N, 2, cpb)
        xy_masked_s[:, :, :, off:off + cpb, :] = blk.transpose(2, 3, 0, 4, 1)
    inv_full = xy_masked_s.reshape(N, 2, HW)[inv]
    xy_masked = np.ascontiguousarray(inv_full).reshape(N, 2, H, W)

    return q_agg, s_agg, z_agg, xy_masked


# revision 9
# speedup vs baseline: 1.1754x; 1.1754x over previous
"""Trainium2 Bass kernel for nn_AggregationLayer (segment_reduce).

Strategy: shard the spatial HW dimension (480*480 = 230400 pixels) across 8
NeuronCores (28800 pixels each). Each core:
  - computes partial masked sums for ALL 64 instances x (8 map channels x 8
    samples + ones) via PE matmul: maskT[hw,64].T @ maps[hw,130] accumulated
    over 225 chunks of 128 pixels (PSUM fp32). Maps are sent as bf16 hi+lo
    pairs (fp32 split) so the bf16 matmul reaches ~fp32 accuracy; masks are
    0/1 which bf16 represents exactly.
  - computes the dense xy_masked product on DVE with broadcast access
    patterns (mask bcast over channel, xy bcast over instance).
Host: pre-arranges inputs per core (pure layout transforms), then sums the 8
partial results, applies mean / L2-normalize / exp (64x9 scalars), and
reassembles xy_masked. No cross-device communication needed.

The 225 chunks are processed in asymmetric blocks (small first block so
compute starts early; small last block to shrink the serial tail), and each
block's store is split into two half-tiles so the output DMA of half 0
overlaps the product of half 1.
"""

import os
from contextlib import ExitStack

import numpy as np
import ml_dtypes

import concourse.bass as bass
import concourse.bacc as bacc
import concourse.tile as tile
import concourse.mybir as mybir
from concourse.bass_utils import run_bass_kernel_spmd

# Problem shape (hardcoded per contract)
B = 8            # samples
N = 64           # instances
NH = N // 2      # half split for stores
H = W = 480
HW = H * W       # 230400
NCORES = 8
HW_CORE = HW // NCORES          # 28800
P = 128                          # partitions
CHUNKS = HW_CORE // P            # 225
BLOCK_SIZES = [30, 55, 55, 55, 20, 10]
assert sum(BLOCK_SIZES) == CHUNKS
BLOCK_OFFS = np.concatenate([[0], np.cumsum(BLOCK_SIZES)[:-1]]).tolist()
NRED = B * 8 + 1                 # 65: (8 samples x [quat4, scales3, z1]) + ones
NCOLS = 2 * NRED                 # 130: hi + lo
NXY = B * 2                      # 16 xy rows
EPS = 1e-12

LAST_RESULT = None  # BassKernelResults of the most recent run (for test harness)

_PROGRAM_CACHE = {}


def _build_program(counts, starts):
    """Build the SPMD Bass program. counts/starts: per-sample instance group
    boundaries in the sorted instance order (baked into DVE APs)."""
    nc = bacc.Bacc(
        "TRN2",
        target_bir_lowering=False,
        debug=False,
        enable_asserts=False,
        num_devices=NCORES,
    )
    bf16 = mybir.dt.bfloat16
    f32 = mybir.dt.float32
    fp8 = mybir.dt.float8e4

    d_mask = nc.dram_tensor("maskt", [P, N * CHUNKS], bf16, kind="ExternalInput").ap()
    d_red = nc.dram_tensor("mapsred", [P, NCOLS * CHUNKS], bf16, kind="ExternalInput").ap()
    d_xy = nc.dram_tensor("mapsxy", [P, NXY * CHUNKS], f32, kind="ExternalInput").ap()
    d_part = nc.dram_tensor("partials", [N, NCOLS], f32, kind="ExternalOutput").ap()
    d_out = nc.dram_tensor("xyout", [P, N * 2 * CHUNKS], f32, kind="ExternalOutput").ap()

    # (sample, n-range) pieces per store half: list of (s, st, cnt) with the
    # sorted-instance range clipped to each half
    halves = []
    for h in range(2):
        lo, hi = h * NH, (h + 1) * NH
        pieces = []
        for s in range(B):
            st, cnt = int(starts[s]), int(counts[s])
            a, b_ = max(st, lo), min(st + cnt, hi)
            if a < b_:
                pieces.append((s, a, b_ - a))
        halves.append(pieces)

    with tile.TileContext(nc) as tc:
        with ExitStack() as ctx:
            pool = ctx.enter_context(tc.tile_pool(name="work", bufs=4))
            opool = ctx.enter_context(tc.tile_pool(name="outs", bufs=2))
            ppool = ctx.enter_context(tc.tile_pool(name="acc", bufs=1, space="PSUM"))

            t_psum = ppool.tile([N, NCOLS], f32)
            nblocks = len(BLOCK_SIZES)
            for b, (cpb, off) in enumerate(zip(BLOCK_SIZES, BLOCK_OFFS)):
                t_mask = pool.tile([P, N * cpb], bf16, tag="mask")
                nc.sync.dma_start(
                    t_mask[:].rearrange("p (n j) -> p n j", n=N),
                    d_mask[:, N * off:N * (off + cpb)].rearrange("p (n j) -> p n j", n=N),
                )
                t_xy = pool.tile([P, NXY * cpb], f32, tag="xy")
                nc.sync.dma_start(t_xy[:], d_xy[:, NXY * off:NXY * (off + cpb)])
                t_red = pool.tile([P, NCOLS * cpb], bf16, tag="red")
                nc.sync.dma_start(t_red[:], d_red[:, NCOLS * off:NCOLS * (off + cpb)])

                # mask is (n j) layout; lhsT per chunk j reads columns at stride cpb
                mask_mm = t_mask[:].rearrange("p (n j) -> p j n", n=N)
                for j in range(cpb):
                    nc.tensor.matmul(
                        t_psum[:],
                        mask_mm[:, j, :],
                        t_red[:, j * NCOLS:(j + 1) * NCOLS],
                        start=(b == 0 and j == 0),
                        stop=(b == nblocks - 1 and j == cpb - 1),
                    )

                mask_r = t_mask[:].rearrange("p (n j) -> p n j", n=N)
                xy_r = t_xy[:].rearrange("p (g j) -> p g j", j=cpb)
                for h in range(2):
                    t_out = opool.tile([P, NH * 2 * cpb], f32, tag=f"out{h}")
                    out_r = t_out[:].rearrange("p (n c j) -> p n c j", n=NH, c=2)
                    for s, a, cnt in halves[h]:
                        ah = a - h * NH
                        o = out_r[:, ah:ah + cnt, :, :]
                        m = mask_r[:, a:a + cnt, :].unsqueeze(2).broadcast_to([P, cnt, 2, cpb])
                        x = xy_r[:, 2 * s:2 * s + 2, :].unsqueeze(1).broadcast_to([P, cnt, 2, cpb])
                        nc.vector.tensor_mul(o, m, x)
                    base = 2 * (N * off + h * NH * cpb)
                    nc.scalar.dma_start(d_out[:, base:base + NH * 2 * cpb], t_out[:])

            t_part = opool.tile([N, NCOLS], f32, tag="part")
            nc.vector.tensor_copy(t_part[:], t_psum[:])
            nc.scalar.dma_start(d_part[:], t_part[:])

    nc.compile()
    return nc


def _blocked_cols(arr_pcj, width):
    """arr_pcj: [P, rows, 225] -> [P, sum(rows*cpb)] with per-block (row, j)
    column layout, blocks in BLOCK_SIZES order. width = rows."""
    outs = []
    for cpb, off in zip(BLOCK_SIZES, BLOCK_OFFS):
        blk = arr_pcj[:, :, off:off + cpb]              # [P, rows, cpb]
        outs.append(blk.reshape(P, width * cpb))
    return np.concatenate(outs, axis=1)


def kernel(instance_masks, sample_ids, quaternion, scales, xy, z):
    global LAST_RESULT
    instance_masks = np.asarray(instance_masks, dtype=np.float32)
    sample_ids = np.asarray(sample_ids).astype(np.int64)
    quaternion = np.asarray(quaternion, dtype=np.float32)
    scales = np.asarray(scales, dtype=np.float32)
    xy = np.asarray(xy, dtype=np.float32)
    z = np.asarray(z, dtype=np.float32)

    # --- host-side layout prep -------------------------------------------
    order = np.argsort(sample_ids, kind="stable")          # n_sorted -> n_orig
    counts = np.bincount(sample_ids, minlength=B)
    starts = np.concatenate([[0], np.cumsum(counts)[:-1]])

    masks_s = instance_masks[order].reshape(N, HW)
    # [core, p, n, 225chunks]
    mask_c = masks_s.astype(ml_dtypes.bfloat16).reshape(N, NCORES, CHUNKS, P)
    mask_c = np.ascontiguousarray(mask_c.transpose(1, 3, 0, 2))   # [core, P, N, 225]

    # reduce maps: [s, ch8, HW] with ch = quat0..3, scale0..2, z -> col s*8+ch
    red = np.concatenate(
        [quaternion.reshape(B, 4, HW), scales.reshape(B, 3, HW), z.reshape(B, 1, HW)],
        axis=1,
    ).reshape(B * 8, HW)
    red65 = np.concatenate([red, np.ones((1, HW), np.float32)], axis=0)  # [65, HW]
    hi = red65.astype(ml_dtypes.bfloat16)
    lo = (red65 - hi.astype(np.float32)).astype(ml_dtypes.bfloat16)
    red130 = np.concatenate([hi, lo], axis=0)              # [130, HW] bf16
    # per-chunk layout is (j, col130): [core, p, 225, 130] -> treat rows=(j col)
    red_c = red130.reshape(NCOLS, NCORES, CHUNKS, P)
    red_c = np.ascontiguousarray(red_c.transpose(1, 3, 2, 0))     # [core, P, 225, 130]

    xy_c = xy.reshape(NXY, NCORES, CHUNKS, P)
    xy_c = np.ascontiguousarray(xy_c.transpose(1, 3, 0, 2))       # [core, P, 16, 225]

    in_maps = []
    for k in range(NCORES):
        maskt = _blocked_cols(mask_c[k], N)
        # red blocks: layout (j, col): reshape [P, 225, 130] slice then flatten
        red_blocks = []
        for cpb, off in zip(BLOCK_SIZES, BLOCK_OFFS):
            red_blocks.append(red_c[k][:, off:off + cpb, :].reshape(P, cpb * NCOLS))
        mapsred = np.concatenate(red_blocks, axis=1)
        mapsxy = _blocked_cols(xy_c[k], NXY)
        in_maps.append({
            "maskt": np.ascontiguousarray(maskt),
            "mapsred": np.ascontiguousarray(mapsred),
            "mapsxy": np.ascontiguousarray(mapsxy),
        })

    # --- build + run ------------------------------------------------------
    key = (tuple(counts.tolist()),)
    if key not in _PROGRAM_CACHE:
        _PROGRAM_CACHE[key] = _build_program(counts, starts)
    nc = _PROGRAM_CACHE[key]

    trace = bool(int(os.environ.get("KERNEL_TRACE", "0")))
    res = run_bass_kernel_spmd(nc, in_maps, core_ids=list(range(NCORES)), trace=trace)
    LAST_RESULT = res

    # --- host-side combine ------------------------------------------------
    parts = np.stack([res.results[k]["partials"] for k in range(NCORES)])  # [8, N, 130]
    S = parts.astype(np.float64).sum(axis=0)
    S = S[:, :NRED] + S[:, NRED:]                                          # [N, 65]

    sid_sorted = sample_ids[order]
    msize = S[:, B * 8]                                                    # [N]
    cols = sid_sorted[:, None] * 8 + np.arange(8)[None, :]                 # [N, 8]
    sums = np.take_along_axis(S[:, :B * 8], cols, axis=1)                  # [N, 8]
    means = sums / msize[:, None]
    q = means[:, :4]
    qn = np.linalg.norm(q, axis=1, keepdims=True)
    q_agg_s = (q / np.maximum(qn, EPS)).astype(np.float32)
    s_agg_s = means[:, 4:7].astype(np.float32)
    z_agg_s = np.exp(means[:, 7:8]).astype(np.float32)

    inv = np.empty(N, dtype=np.int64)
    inv[order] = np.arange(N)
    q_agg = q_agg_s[inv]
    s_agg = s_agg_s[inv]
    z_agg = z_agg_s[inv]

    # xyout cols: per block, (n, c, j) with n split in halves handled by layout
    X = np.stack([res.results[k]["xyout"] for k in range(NCORES)])  # [8, P, N*2*225]
    xy_masked_s = np.empty((N, 2, NCORES, CHUNKS, P), np.float32)
    for cpb, off in zip(BLOCK_SIZES, BLOCK_OFFS):
        base = 2 * N * off
        blk = X[:, :, base:base + N * 2 * cpb].reshape(NCORES, P, N, 2, cpb)
        xy_masked_s[:, :, :, off:off + cpb, :] = blk.transpose(2, 3, 0, 4, 1)
    inv_full = xy_masked_s.reshape(N, 2, HW)[inv]
    xy_masked = np.ascontiguousarray(inv_full).reshape(N, 2, H, W)

    return q_agg, s_agg, z_agg, xy_masked
